# revision 45
# baseline (speedup 1.0000x reference)
"""GQA attention (B=2, T=2048, H=2048, 16 heads / 4 kv heads, RoPE, causal)
distributed over 8 trn2 NeuronCores.

Sharding: 2 q-heads + their kv head per core (tensor parallel). Each core
computes q/k/v projections for its heads, RoPE, causal attention, then
per-head AllToAlls reshard attention output from head-parallel to
token-parallel so the o-projection runs against the full wo with no
AllReduce; each core returns its own 512 tokens of the final output and
the host concatenates.

Matmuls run in float32r (full-rate fp32 on the PE, ~12-bit mantissa);
weights and x arrive pre-rounded to the f32r grid so they DMA straight
into matmul operands with no cast instructions. Scores are computed
transposed (sT[k,q]) so softmax+PV need no transposes; row sums come from
a ones-vector matmul; exp runs without max-subtraction (scores are O(1),
fp32 exp cannot overflow). The attention inner loop is software-pipelined
(scores 2 k-tiles ahead of PV). Phases are fused: both batches project
back-to-back, then both attend back-to-back; the o-projection of batch 0
and the wo prefetch ride inside batch 1's attention as fillers.
"""

import sys

for _p in ("/opt/trn_rl_repo", "/root/.axon_site/_ro/trn_rl_repo"):
    if _p not in sys.path:
        sys.path.append(_p)

import numpy as np

import concourse.bacc as bacc
import concourse.mybir as mybir
import concourse.tile as tile
from concourse.bass_utils import run_bass_kernel_spmd

B, T, H = 2, 2048, 2048
N_HEAD, N_KV_HEAD = 16, 4
HD = H // N_HEAD  # 128
TOK = B * T  # 4096
CORES = 8
HPC = N_HEAD // CORES  # 2 q heads per core
DPC = HPC * HD  # 256 q dims per core
OWN = T // CORES  # 256 own tokens per batch per core
SCALE = HD**-0.5
ROPE_THETA = 10000.0

F32 = mybir.dt.float32
F32R = mybir.dt.float32r
BF16 = mybir.dt.bfloat16
EXP = mybir.ActivationFunctionType.Exp

_CACHE = {}


def _build_nc():
    nc = bacc.Bacc("TRN2", target_bir_lowering=False, debug=False, num_devices=CORES)

    x_e = nc.declare_dram_parameter("x", [TOK, H], F32R, isOutput=False)
    wq_e = nc.declare_dram_parameter("wq", [H, DPC], F32R, isOutput=False)
    wk_e = nc.declare_dram_parameter("wk", [H, HD], F32R, isOutput=False)
    wv_e = nc.declare_dram_parameter("wv", [H, HD], F32R, isOutput=False)
    wo_e = nc.declare_dram_parameter("wo", [H, H], BF16, isOutput=False)
    cos_e = nc.declare_dram_parameter("cosT", [HD, T], F32, isOutput=False)
    sin_e = nc.declare_dram_parameter("sinT", [HD, T], F32, isOutput=False)
    msk_e = nc.declare_dram_parameter("dmask", [128, 4 * 512], F32, isOutput=False)
    id_e = nc.declare_dram_parameter("ident", [128, 128], F32R, isOutput=False)
    out_e = nc.declare_dram_parameter("out", [B, OWN, H], F32, isOutput=True)

    with tile.TileContext(nc) as tc:
        with (
            tc.tile_pool(name="dram", bufs=1, space="DRAM") as dpool,
            tc.tile_pool(name="const", bufs=1) as cpool,
        ):
            send = [
                [
                    dpool.tile(
                        [CORES, HD, OWN], BF16, name=f"send{b}h{h}", tag=f"send{b}h{h}"
                    )
                    for h in range(HPC)
                ]
                for b in range(B)
            ]
            recv = [
                [
                    dpool.tile(
                        [CORES, HD, OWN], BF16, name=f"recv{b}h{h}", tag=f"recv{b}h{h}"
                    )
                    for h in range(HPC)
                ]
                for b in range(B)
            ]

            ident = cpool.tile([128, 128], F32R)
            nc.sync.dma_start(out=ident[:, :], in_=id_e[:, :])
            cosT = cpool.tile([HD, T], F32)
            sinT = cpool.tile([HD, T], F32)
            dmask = cpool.tile([128, 4 * 512], F32)

            ones_f32 = cpool.tile([128, 1], F32)
            nc.vector.memset(ones_f32[:, :], 1.0)
            ones_col = cpool.tile([128, 1], F32R)
            nc.vector.tensor_copy(ones_col[:, :], ones_f32[:, :])

            wq_r = cpool.tile([128, 16, DPC], F32R)
            wk_r = cpool.tile([128, 16, HD], F32R)
            wv_r = cpool.tile([128, 16, HD], F32R)

            def emit_const_loads():
                nc.sync.dma_start(out=cosT[:, :], in_=cos_e[:, :])
                nc.sync.dma_start(out=sinT[:, :], in_=sin_e[:, :])
                nc.sync.dma_start(out=dmask[:, :], in_=msk_e[:, :])
                nc.sync.dma_start(
                    out=wq_r[:, :, :], in_=wq_e.rearrange("(k p) c -> p k c", p=128)
                )
                nc.sync.dma_start(
                    out=wk_r[:, :, :], in_=wk_e.rearrange("(k p) c -> p k c", p=128)
                )
                nc.sync.dma_start(
                    out=wv_r[:, :, :], in_=wv_e.rearrange("(k p) c -> p k c", p=128)
                )

            # q/k/v activations for BOTH batches stay resident
            qkv = tc.tile_pool(name="qkv", bufs=1)
            qkvp = qkv.__enter__()
            qTr = [
                qkvp.tile([128, HPC, T], F32R, name=f"qTr{b}", tag=f"qTr{b}")
                for b in range(B)
            ]
            kTr = [
                qkvp.tile([128, T], F32R, name=f"kTr{b}", tag=f"kTr{b}")
                for b in range(B)
            ]
            vtm = [
                qkvp.tile([128, 16, HD], F32R, name=f"vtm{b}", tag=f"vtm{b}")
                for b in range(B)
            ]

            # ---- projection of both batches, one continuous phase ----
            with (
                tc.tile_pool(name="px", bufs=1) as px,
                tc.tile_pool(name="ppj", bufs=2, space="PSUM") as ppsum,
            ):
                for b in range(B):
                    for ch in range(4):
                        row0 = b * T + ch * 512
                        xT = px.tile(
                            [128, 4, 16, 128], F32R, tag="xT", bufs=1, name="xT"
                        )
                        for tt in range(4):
                            xin = px.tile(
                                [128, 2048], F32R, tag="xin", bufs=4, name="xin"
                            )
                            nc.sync.dma_start(
                                out=xin[:, :],
                                in_=x_e[row0 + tt * 128 : row0 + (tt + 1) * 128, :],
                            )
                            for fb in range(4):
                                bank = ppsum.tile(
                                    [128, 512], F32R, tag="tr", name="trbank"
                                )
                                for j in range(4):
                                    ft = fb * 4 + j
                                    nc.tensor.transpose(
                                        bank[:, j * 128 : (j + 1) * 128],
                                        xin[:, ft * 128 : (ft + 1) * 128],
                                        ident[:, :],
                                    )
                                if fb % 2 == 0:
                                    nc.vector.tensor_copy(
                                        xT[:, tt, fb * 4 : (fb + 1) * 4, :],
                                        bank[:, :],
                                    )
                                else:
                                    nc.scalar.copy(
                                        xT[:, tt, fb * 4 : (fb + 1) * 4, :],
                                        bank[:, :],
                                    )

                        if b == 0 and ch == 0:
                            emit_const_loads()

                        tsl = slice(ch * 512, (ch + 1) * 512)

                        def rope(dst, ps):
                            # the very last chunk's rope gates the proj->attn
                            # PSUM pool handoff: run its muls on idle gpsimd
                            eng = nc.gpsimd if (b == 1 and ch == 3) else nc.vector
                            sb = px.tile([128, 512], F32, tag="rsb", bufs=2, name="rsb")
                            nc.vector.tensor_copy(sb[:, :], ps[:, :])
                            rot = px.tile(
                                [128, 512], F32, tag="rrot", bufs=2, name="rrot"
                            )
                            nc.gpsimd.dma_start(out=rot[0:64, :], in_=sb[64:128, :])
                            nc.gpsimd.dma_start(out=rot[64:128, :], in_=sb[0:64, :])
                            t1 = px.tile([128, 512], F32, tag="rt1", bufs=2, name="rt1")
                            eng.tensor_mul(t1[:, :], sb[:, :], cosT[:, tsl])
                            t2 = px.tile([128, 512], F32, tag="rt2", bufs=2, name="rt2")
                            eng.tensor_mul(t2[:, :], rot[:, :], sinT[:, tsl])
                            eng.tensor_add(dst, t1[:, :], t2[:, :])

                        for m in range(HPC):
                            psq = ppsum.tile([128, 512], F32, tag="pj", name="psq")
                            for k in range(16):
                                nc.tensor.matmul(
                                    psq[:, :],
                                    wq_r[:, k, m * 128 : (m + 1) * 128],
                                    xT[:, :, k, :],
                                    start=(k == 0),
                                    stop=(k == 15),
                                )
                            rope(qTr[b][:, m, tsl], psq)

                        psk = ppsum.tile([128, 512], F32, tag="pj", name="psk")
                        for k in range(16):
                            nc.tensor.matmul(
                                psk[:, :],
                                wk_r[:, k, :],
                                xT[:, :, k, :],
                                start=(k == 0),
                                stop=(k == 15),
                            )
                        rope(kTr[b][:, tsl], psk)

                        psv = ppsum.tile([128, 512], F32, tag="pj", name="psv")
                        for k in range(16):
                            nc.tensor.matmul(
                                psv[:, :],
                                wv_r[:, k, :],
                                xT[:, :, k, :],
                                start=(k == 0),
                                stop=(k == 15),
                            )
                        vsb = px.tile([128, 512], F32R, tag="vsb", bufs=2, name="vsb")
                        nc.vector.tensor_copy(vsb[:, :], psv[:, :])
                        vbank = ppsum.tile([128, 512], F32R, tag="tr", name="vbank")
                        for s in range(4):
                            nc.tensor.transpose(
                                vbank[:, s * 128 : (s + 1) * 128],
                                vsb[:, s * 128 : (s + 1) * 128],
                                ident[:, :],
                            )
                        nc.vector.tensor_copy(
                            vtm[b][:, ch * 4 : (ch + 1) * 4, :], vbank[:, :]
                        )

            # ---- o-projection pieces (emitted inline or as fillers) ----
            wopre_holder = {}

            def emit_oproj(b, opool, opsum_pool, psum_bufs=2, use_prefetch=False):
                ao = opool.tile([128, 16, OWN], BF16, name=f"aor{b}", tag=f"aor{b}")
                ems = []

                def ao_em():
                    for hh in range(2):
                        nc.sync.dma_start(
                            out=ao[:, hh::2, :],
                            in_=recv[b][hh].rearrange("s p t -> p s t"),
                        )

                ems.append(ao_em)
                for nb in range(4):

                    def nb_em(nb=nb):
                        nsl = slice(nb * 512, (nb + 1) * 512)
                        if use_prefetch and nb >= 2:
                            # refill the slot nb-2 finished with
                            nc.sync.dma_start(
                                out=wopre_holder["t"][:, nb - 2, :, :],
                                in_=wo_e[:, nsl].rearrange("(k p) c -> p k c", p=128),
                            )
                        n_groups = 1 if psum_bufs == 1 else 2
                        for g in range(0, 2, n_groups):
                            pss_ = [
                                opsum_pool.tile(
                                    [128, 512],
                                    F32,
                                    tag=f"op{b}",
                                    bufs=psum_bufs,
                                    name="opsb",
                                )
                                for _ in range(n_groups)
                            ]
                            for k in range(16):
                                if use_prefetch:
                                    wr_ap = wopre_holder["t"][:, nb % 2, k, :]
                                else:
                                    wrt = opool.tile(
                                        [128, 512],
                                        BF16,
                                        tag=f"wr{b}",
                                        bufs=8,
                                        name="wr",
                                    )
                                    nc.sync.dma_start(
                                        out=wrt[:, :],
                                        in_=wo_e[k * 128 : (k + 1) * 128, nsl],
                                    )
                                    wr_ap = wrt[:, :]
                                for gi in range(n_groups):
                                    tt = g + gi
                                    nc.tensor.matmul(
                                        pss_[gi][:, :],
                                        ao[:, k, tt * 128 : (tt + 1) * 128],
                                        wr_ap,
                                        start=(k == 0),
                                        stop=(k == 15),
                                    )
                            for gi in range(n_groups):
                                tt = g + gi
                                osb = opool.tile(
                                    [128, 512], F32, tag=f"osb{b}", bufs=4, name="osb"
                                )
                                nc.vector.tensor_copy(osb[:, :], pss_[gi][:, :])
                                nc.sync.dma_start(
                                    out=out_e[b, tt * 128 : (tt + 1) * 128, nsl],
                                    in_=osb[:, :],
                                )

                    ems.append(nb_em)
                return ems

            def emit_attention(b, ap, apsum, fillers=(), fill_from=0):
                fill = list(fillers)
                fi = 0
                it_idx = -1
                o_bufs = 2
                l_bufs = 1 if fillers else 2
                pt_bufs = 6 if fillers else 8
                et_bufs = 3 if fillers else 4
                for h in range(HPC):
                    for qb in range(4):
                        it_idx += 1
                        qsl = slice(qb * 512, (qb + 1) * 512)
                        nkt = 4 * (qb + 1)
                        pso = apsum.tile([128, 512], F32, tag="o", bufs=o_bufs, name="pso")
                        psl = apsum.tile(
                            [1, 512], F32, tag="l", bufs=l_bufs, name="psl"
                        )
                        pts = {}

                        def score(kt):
                            r = kt - 4 * qb
                            # live columns of a diagonal-band tile start at rs
                            rs = r * 128 if r > 0 else 0
                            pss = apsum.tile(
                                [128, 512], F32, tag="s", bufs=4, name="pss"
                            )
                            nc.tensor.matmul(
                                pss[:, rs:],
                                kTr[b][:, kt * 128 : (kt + 1) * 128],
                                qTr[b][:, h, qb * 512 + rs : (qb + 1) * 512],
                                start=True,
                                stop=True,
                            )
                            pT = ap.tile([128, 512], F32R, tag="pT", bufs=pt_bufs, name="pT")
                            if r >= 0:
                                et = ap.tile(
                                    [128, 512], F32, tag="et", bufs=et_bufs, name="et"
                                )
                                nc.scalar.activation(et[:, rs:], pss[:, rs:], EXP)
                                nc.vector.tensor_mul(
                                    pT[:, rs:],
                                    et[:, rs:],
                                    dmask[:, r * 512 + rs : (r + 1) * 512],
                                )
                            else:
                                nc.scalar.activation(pT[:, :], pss[:, :], EXP)
                            pts[kt] = (pT, rs)

                        def pv(kt):
                            pT, rs = pts.pop(kt)
                            nc.tensor.matmul(
                                pso[:, rs:],
                                vtm[b][:, kt, :],
                                pT[:, rs:],
                                start=(kt == 0),
                                stop=(kt == nkt - 1),
                            )
                            nc.tensor.matmul(
                                psl[0:1, rs:],
                                ones_col[:, :],
                                pT[:, rs:],
                                start=(kt == 0),
                                stop=(kt == nkt - 1),
                            )

                        LA = 2  # scores run this many k-tiles ahead of PV
                        for kt in range(nkt):
                            score(kt)
                            if kt >= LA:
                                pv(kt - LA)
                        for kt in range(max(0, nkt - LA), nkt):
                            pv(kt)

                        linv = ap.tile([1, 512], F32, tag="li", bufs=2, name="linv")
                        nc.vector.reciprocal(linv[:, :], psl[0:1, :])
                        lbc = ap.tile([128, 512], F32, tag="lbc", bufs=2, name="lbc")
                        nc.gpsimd.partition_broadcast(lbc[:, :], linv[0:1, :])
                        aout = ap.tile([128, 512], BF16, tag="ao", bufs=2, name="aout")
                        nc.vector.tensor_mul(aout[:, :], pso[:, :], lbc[:, :])
                        for half in range(2):
                            j = 2 * qb + half
                            nc.sync.dma_start(
                                out=send[b][h][j, :, :],
                                in_=aout[:, half * 256 : (half + 1) * 256],
                            )
                        if fi < len(fill) and it_idx >= fill_from:
                            fill[fi]()
                            fi += 1
                    nc.gpsimd.collective_compute(
                        "AllToAll",
                        mybir.AluOpType.bypass,
                        replica_groups=[list(range(CORES))],
                        ins=[send[b][h].opt()],
                        outs=[recv[b][h].opt()],
                    )
                for i in range(fi, len(fill)):
                    fill[i]()

            # ---- attention, batch 0 then batch 1 ----
            with tc.tile_pool(name="wop", bufs=1) as wop:
                wopre_holder["t"] = wop.tile([128, 2, 16, 512], BF16, name="wopre")

                def emit_wo_prefetch():
                    for nb in range(2):
                        nc.sync.dma_start(
                            out=wopre_holder["t"][:, nb, :, :],
                            in_=wo_e[:, nb * 512 : (nb + 1) * 512].rearrange(
                                "(k p) c -> p k c", p=128
                            ),
                        )

                with (
                    tc.tile_pool(name="ap0", bufs=2, space="PSUM") as apsum0,
                    tc.tile_pool(name="ab0", bufs=1) as ab0,
                ):
                    emit_attention(0, ab0, apsum0)
                emit_wo_prefetch()

                with (
                    tc.tile_pool(name="ap1", bufs=2, space="PSUM") as apsum1,
                    tc.tile_pool(name="ab1", bufs=1) as ab1,
                    tc.tile_pool(name="op0", bufs=1) as op0,
                    tc.tile_pool(name="ops0", bufs=1, space="PSUM") as ops0,
                ):
                    ems = emit_oproj(0, op0, ops0, psum_bufs=1)
                    emit_attention(1, ab1, apsum1, fillers=ems, fill_from=3)

                # ---- o-projection for batch 1 ----
                with (
                    tc.tile_pool(name="op1", bufs=1) as op1,
                    tc.tile_pool(name="ops1p", bufs=1, space="PSUM") as ops1,
                ):
                    for em in emit_oproj(1, op1, ops1, use_prefetch=True):
                        em()

            qkv.__exit__(None, None, None)

    nc.compile()
    return nc


def _host_tables():
    inv_freq = 1.0 / (ROPE_THETA ** (np.arange(0, HD, 2, dtype=np.float64) / HD))
    pos = np.arange(T, dtype=np.float64)
    freqs = pos[:, None] * inv_freq[None, :]  # [T, 64]
    emb = np.concatenate([freqs, freqs], axis=-1)  # [T, 128]
    cosT = np.cos(emb).T.astype(np.float32)  # [128, T]
    sinT = np.sin(emb).T.astype(np.float32)
    sinT[:64, :] *= -1.0  # sign of the rotate-half fold
    # diagonal-band causal masks: dmask[r][k', q'] = 1 if q' >= 128 r + k'
    q = np.arange(512)[None, :]
    kk = np.arange(128)[:, None]
    dm = np.concatenate(
        [(q >= 128 * r + kk).astype(np.float32) for r in range(4)], axis=1
    )  # [128, 2048]
    ident = np.eye(128, dtype=np.float32)
    return cosT, sinT, dm, ident


def _pre_round(a):
    b = np.ascontiguousarray(a, dtype=np.float32).view(np.uint32)
    return ((b + np.uint32(0x800)) & np.uint32(0xFFFFF000)).view(np.float32)


def _run(inputs, trace=False):
    if "nc" not in _CACHE:
        _CACHE["nc"] = _build_nc()
    nc = _CACHE["nc"]

    import ml_dtypes

    x = _pre_round(np.asarray(inputs["x"], dtype=np.float32).reshape(TOK, H))
    wq = _pre_round(np.asarray(inputs["wq"], dtype=np.float32) * np.float32(SCALE))
    wk = _pre_round(np.asarray(inputs["wk"], dtype=np.float32))
    wv = _pre_round(np.asarray(inputs["wv"], dtype=np.float32))
    wo = np.asarray(inputs["wo"], dtype=np.float32).astype(ml_dtypes.bfloat16)
    cosT, sinT, dm, ident = _host_tables()

    in_maps = []
    for c in range(CORES):
        kv = c // 2
        in_maps.append(
            {
                "x": x,
                "wq": np.ascontiguousarray(wq[:, c * DPC : (c + 1) * DPC]),
                "wk": np.ascontiguousarray(wk[:, kv * HD : (kv + 1) * HD]),
                "wv": np.ascontiguousarray(wv[:, kv * HD : (kv + 1) * HD]),
                "wo": wo,
                "cosT": cosT,
                "sinT": sinT,
                "dmask": dm,
                "ident": ident,
            }
        )

    res = run_bass_kernel_spmd(nc, in_maps, core_ids=list(range(CORES)), trace=trace)
    out = np.empty((B, T, H), dtype=np.float32)
    for c in range(CORES):
        o = res.results[c]["out"]  # [B, OWN, H]
        for b in range(B):
            out[b, c * OWN : (c + 1) * OWN, :] = o[b]
    return out, res


def _run_subprocess(inputs):
    """Fresh-process fallback for transient device faults."""
    import os
    import subprocess
    import tempfile

    d = tempfile.mkdtemp()
    inp = os.path.join(d, "in.npz")
    outp = os.path.join(d, "out.npy")
    np.savez(inp, **{k: np.asarray(v) for k, v in inputs.items()})
    code = (
        "import sys, numpy as np; "
        f"sys.path.insert(0, {os.path.dirname(os.path.abspath(__file__))!r}); "
        "import kernel as K; "
        f"d = np.load({inp!r}); "
        "out = K.kernel(**{k: d[k] for k in d.files}); "
        f"np.save({outp!r}, out)"
    )
    subprocess.run([sys.executable, "-c", code], check=True, timeout=900)
    return np.load(outp)


def kernel(**inputs) -> np.ndarray:
    try:
        out, _ = _run(inputs, trace=False)
        return out
    except Exception:
        pass
    # transient accelerator fault: retry in-process once, then isolate
    try:
        out, _ = _run(inputs, trace=False)
        return out
    except Exception:
        pass
    for attempt in range(2):
        try:
            return _run_subprocess(inputs)
        except Exception:
            if attempt == 1:
                raise
    raise RuntimeError("unreachable")


# revision 46
# speedup vs baseline: 1.0105x; 1.0105x over previous
"""GQA attention (B=2, T=2048, H=2048, 16 heads / 4 kv heads, RoPE, causal)
distributed over 8 trn2 NeuronCores.

Sharding: 2 q-heads + their kv head per core (tensor parallel). Each core
computes q/k/v projections for its heads, RoPE, causal attention, then
per-head AllToAlls reshard attention output from head-parallel to
token-parallel so the o-projection runs against the full wo with no
AllReduce; each core returns its own 512 tokens of the final output and
the host concatenates.

Matmuls run in float32r (full-rate fp32 on the PE, ~12-bit mantissa);
weights and x arrive pre-rounded to the f32r grid so they DMA straight
into matmul operands with no cast instructions. Scores are computed
transposed (sT[k,q]) so softmax+PV need no transposes; row sums come from
a ones-vector matmul; exp runs without max-subtraction (scores are O(1),
fp32 exp cannot overflow). The attention inner loop is software-pipelined
(scores 2 k-tiles ahead of PV). Phases are fused: both batches project
back-to-back, then both attend back-to-back; the o-projection of batch 0
and the wo prefetch ride inside batch 1's attention as fillers.
"""

import sys

for _p in ("/opt/trn_rl_repo", "/root/.axon_site/_ro/trn_rl_repo"):
    if _p not in sys.path:
        sys.path.append(_p)

import numpy as np

import concourse.bacc as bacc
import concourse.mybir as mybir
import concourse.tile as tile
from concourse.bass_utils import run_bass_kernel_spmd

B, T, H = 2, 2048, 2048
N_HEAD, N_KV_HEAD = 16, 4
HD = H // N_HEAD  # 128
TOK = B * T  # 4096
CORES = 8
HPC = N_HEAD // CORES  # 2 q heads per core
DPC = HPC * HD  # 256 q dims per core
OWN = T // CORES  # 256 own tokens per batch per core
SCALE = HD**-0.5
ROPE_THETA = 10000.0

F32 = mybir.dt.float32
F32R = mybir.dt.float32r
BF16 = mybir.dt.bfloat16
EXP = mybir.ActivationFunctionType.Exp

_CACHE = {}


def _build_nc():
    nc = bacc.Bacc("TRN2", target_bir_lowering=False, debug=False, num_devices=CORES)

    x_e = nc.declare_dram_parameter("x", [TOK, H], F32R, isOutput=False)
    wq_e = nc.declare_dram_parameter("wq", [H, DPC], F32R, isOutput=False)
    wk_e = nc.declare_dram_parameter("wk", [H, HD], F32R, isOutput=False)
    wv_e = nc.declare_dram_parameter("wv", [H, HD], F32R, isOutput=False)
    wo_e = nc.declare_dram_parameter("wo", [H, H], BF16, isOutput=False)
    cos_e = nc.declare_dram_parameter("cosT", [HD, T], F32, isOutput=False)
    sin_e = nc.declare_dram_parameter("sinT", [HD, T], F32, isOutput=False)
    msk_e = nc.declare_dram_parameter("dmask", [128, 4 * 512], F32, isOutput=False)
    id_e = nc.declare_dram_parameter("ident", [128, 128], F32R, isOutput=False)
    out_e = nc.declare_dram_parameter("out", [B, OWN, H], F32, isOutput=True)

    with tile.TileContext(nc) as tc:
        with (
            tc.tile_pool(name="dram", bufs=1, space="DRAM") as dpool,
            tc.tile_pool(name="const", bufs=1) as cpool,
        ):
            send = [
                [
                    dpool.tile(
                        [CORES, HD, OWN], BF16, name=f"send{b}h{h}", tag=f"send{b}h{h}"
                    )
                    for h in range(HPC)
                ]
                for b in range(B)
            ]
            recv = [
                [
                    dpool.tile(
                        [CORES, HD, OWN], BF16, name=f"recv{b}h{h}", tag=f"recv{b}h{h}"
                    )
                    for h in range(HPC)
                ]
                for b in range(B)
            ]

            ident = cpool.tile([128, 128], F32R)
            nc.sync.dma_start(out=ident[:, :], in_=id_e[:, :])
            cosT = cpool.tile([HD, T], F32)
            sinT = cpool.tile([HD, T], F32)
            dmask = cpool.tile([128, 4 * 512], F32)

            ones_f32 = cpool.tile([128, 1], F32)
            nc.vector.memset(ones_f32[:, :], 1.0)
            ones_col = cpool.tile([128, 1], F32R)
            nc.vector.tensor_copy(ones_col[:, :], ones_f32[:, :])

            wq_r = cpool.tile([128, 16, DPC], F32R)
            wk_r = cpool.tile([128, 16, HD], F32R)
            wv_r = cpool.tile([128, 16, HD], F32R)

            def emit_const_loads():
                nc.sync.dma_start(out=cosT[:, :], in_=cos_e[:, :])
                nc.sync.dma_start(out=sinT[:, :], in_=sin_e[:, :])
                nc.sync.dma_start(out=dmask[:, :], in_=msk_e[:, :])
                nc.sync.dma_start(
                    out=wq_r[:, :, :], in_=wq_e.rearrange("(k p) c -> p k c", p=128)
                )
                nc.sync.dma_start(
                    out=wk_r[:, :, :], in_=wk_e.rearrange("(k p) c -> p k c", p=128)
                )
                nc.sync.dma_start(
                    out=wv_r[:, :, :], in_=wv_e.rearrange("(k p) c -> p k c", p=128)
                )

            # q/k/v activations for BOTH batches stay resident
            qkv = tc.tile_pool(name="qkv", bufs=1)
            qkvp = qkv.__enter__()
            qTr = [
                qkvp.tile([128, HPC, T], F32R, name=f"qTr{b}", tag=f"qTr{b}")
                for b in range(B)
            ]
            kTr = [
                qkvp.tile([128, T], F32R, name=f"kTr{b}", tag=f"kTr{b}")
                for b in range(B)
            ]
            vtm = [
                qkvp.tile([128, 16, HD], F32R, name=f"vtm{b}", tag=f"vtm{b}")
                for b in range(B)
            ]

            # ---- projection of both batches, one continuous phase ----
            with (
                tc.tile_pool(name="px", bufs=1) as px,
                tc.tile_pool(name="ppj", bufs=2, space="PSUM") as ppsum,
            ):
                for b in range(B):
                    for ch in range(4):
                        row0 = b * T + ch * 512
                        xT = px.tile(
                            [128, 4, 16, 128], F32R, tag="xT", bufs=1, name="xT"
                        )
                        for tt in range(4):
                            xin = px.tile(
                                [128, 2048], F32R, tag="xin", bufs=4, name="xin"
                            )
                            nc.sync.dma_start(
                                out=xin[:, :],
                                in_=x_e[row0 + tt * 128 : row0 + (tt + 1) * 128, :],
                            )
                            for fb in range(4):
                                bank = ppsum.tile(
                                    [128, 512], F32R, tag="tr", name="trbank"
                                )
                                for j in range(4):
                                    ft = fb * 4 + j
                                    nc.tensor.transpose(
                                        bank[:, j * 128 : (j + 1) * 128],
                                        xin[:, ft * 128 : (ft + 1) * 128],
                                        ident[:, :],
                                    )
                                if fb % 2 == 0:
                                    nc.vector.tensor_copy(
                                        xT[:, tt, fb * 4 : (fb + 1) * 4, :],
                                        bank[:, :],
                                    )
                                else:
                                    nc.scalar.copy(
                                        xT[:, tt, fb * 4 : (fb + 1) * 4, :],
                                        bank[:, :],
                                    )

                        if b == 0 and ch == 0:
                            emit_const_loads()

                        tsl = slice(ch * 512, (ch + 1) * 512)

                        def rope(dst, ps):
                            sb = px.tile([128, 512], F32, tag="rsb", bufs=2, name="rsb")
                            nc.vector.tensor_copy(sb[:, :], ps[:, :])
                            rot = px.tile(
                                [128, 512], F32, tag="rrot", bufs=2, name="rrot"
                            )
                            nc.gpsimd.dma_start(out=rot[0:64, :], in_=sb[64:128, :])
                            nc.gpsimd.dma_start(out=rot[64:128, :], in_=sb[0:64, :])
                            t1 = px.tile([128, 512], F32, tag="rt1", bufs=2, name="rt1")
                            nc.vector.tensor_mul(t1[:, :], sb[:, :], cosT[:, tsl])
                            t2 = px.tile([128, 512], F32, tag="rt2", bufs=2, name="rt2")
                            nc.vector.tensor_mul(t2[:, :], rot[:, :], sinT[:, tsl])
                            nc.vector.tensor_add(dst, t1[:, :], t2[:, :])

                        for m in range(HPC):
                            psq = ppsum.tile([128, 512], F32, tag="pj", name="psq")
                            for k in range(16):
                                nc.tensor.matmul(
                                    psq[:, :],
                                    wq_r[:, k, m * 128 : (m + 1) * 128],
                                    xT[:, :, k, :],
                                    start=(k == 0),
                                    stop=(k == 15),
                                )
                            rope(qTr[b][:, m, tsl], psq)

                        psk = ppsum.tile([128, 512], F32, tag="pj", name="psk")
                        for k in range(16):
                            nc.tensor.matmul(
                                psk[:, :],
                                wk_r[:, k, :],
                                xT[:, :, k, :],
                                start=(k == 0),
                                stop=(k == 15),
                            )
                        rope(kTr[b][:, tsl], psk)

                        psv = ppsum.tile([128, 512], F32, tag="pj", name="psv")
                        for k in range(16):
                            nc.tensor.matmul(
                                psv[:, :],
                                wv_r[:, k, :],
                                xT[:, :, k, :],
                                start=(k == 0),
                                stop=(k == 15),
                            )
                        vsb = px.tile([128, 512], F32R, tag="vsb", bufs=2, name="vsb")
                        nc.vector.tensor_copy(vsb[:, :], psv[:, :])
                        vbank = ppsum.tile([128, 512], F32R, tag="tr", name="vbank")
                        for s in range(4):
                            nc.tensor.transpose(
                                vbank[:, s * 128 : (s + 1) * 128],
                                vsb[:, s * 128 : (s + 1) * 128],
                                ident[:, :],
                            )
                        nc.vector.tensor_copy(
                            vtm[b][:, ch * 4 : (ch + 1) * 4, :], vbank[:, :]
                        )

            # ---- o-projection pieces (emitted inline or as fillers) ----
            wopre_holder = {}

            def emit_oproj(b, opool, opsum_pool, psum_bufs=2, use_prefetch=False):
                ao = opool.tile([128, 16, OWN], BF16, name=f"aor{b}", tag=f"aor{b}")
                ems = []

                def ao_em():
                    for hh in range(2):
                        nc.sync.dma_start(
                            out=ao[:, hh::2, :],
                            in_=recv[b][hh].rearrange("s p t -> p s t"),
                        )

                ems.append(ao_em)
                for nb in range(4):

                    def nb_em(nb=nb):
                        nsl = slice(nb * 512, (nb + 1) * 512)
                        if use_prefetch and nb >= 2:
                            # refill the slot nb-2 finished with
                            nc.sync.dma_start(
                                out=wopre_holder["t"][:, nb - 2, :, :],
                                in_=wo_e[:, nsl].rearrange("(k p) c -> p k c", p=128),
                            )
                        n_groups = 1 if psum_bufs == 1 else 2
                        for g in range(0, 2, n_groups):
                            pss_ = [
                                opsum_pool.tile(
                                    [128, 512],
                                    F32,
                                    tag=f"op{b}",
                                    bufs=psum_bufs,
                                    name="opsb",
                                )
                                for _ in range(n_groups)
                            ]
                            for k in range(16):
                                if use_prefetch:
                                    wr_ap = wopre_holder["t"][:, nb % 2, k, :]
                                else:
                                    wrt = opool.tile(
                                        [128, 512],
                                        BF16,
                                        tag=f"wr{b}",
                                        bufs=8,
                                        name="wr",
                                    )
                                    nc.sync.dma_start(
                                        out=wrt[:, :],
                                        in_=wo_e[k * 128 : (k + 1) * 128, nsl],
                                    )
                                    wr_ap = wrt[:, :]
                                for gi in range(n_groups):
                                    tt = g + gi
                                    nc.tensor.matmul(
                                        pss_[gi][:, :],
                                        ao[:, k, tt * 128 : (tt + 1) * 128],
                                        wr_ap,
                                        start=(k == 0),
                                        stop=(k == 15),
                                    )
                            for gi in range(n_groups):
                                tt = g + gi
                                osb = opool.tile(
                                    [128, 512], F32, tag=f"osb{b}", bufs=4, name="osb"
                                )
                                nc.vector.tensor_copy(osb[:, :], pss_[gi][:, :])
                                nc.sync.dma_start(
                                    out=out_e[b, tt * 128 : (tt + 1) * 128, nsl],
                                    in_=osb[:, :],
                                )

                    ems.append(nb_em)
                return ems

            def emit_attention(b, ap, apsum, fillers=(), fill_from=0):
                fill = list(fillers)
                fi = 0
                it_idx = -1
                o_bufs = 2
                l_bufs = 1 if fillers else 2
                pt_bufs = 6 if fillers else 8
                et_bufs = 3 if fillers else 4
                for h in range(HPC):
                    for qb in range(4):
                        it_idx += 1
                        qsl = slice(qb * 512, (qb + 1) * 512)
                        nkt = 4 * (qb + 1)
                        pso = apsum.tile([128, 512], F32, tag="o", bufs=o_bufs, name="pso")
                        psl = apsum.tile(
                            [1, 512], F32, tag="l", bufs=l_bufs, name="psl"
                        )
                        pts = {}

                        def score(kt):
                            r = kt - 4 * qb
                            # live columns of a diagonal-band tile start at rs
                            rs = r * 128 if r > 0 else 0
                            pss = apsum.tile(
                                [128, 512], F32, tag="s", bufs=4, name="pss"
                            )
                            nc.tensor.matmul(
                                pss[:, rs:],
                                kTr[b][:, kt * 128 : (kt + 1) * 128],
                                qTr[b][:, h, qb * 512 + rs : (qb + 1) * 512],
                                start=True,
                                stop=True,
                            )
                            pT = ap.tile([128, 512], F32R, tag="pT", bufs=pt_bufs, name="pT")
                            if r >= 0:
                                et = ap.tile(
                                    [128, 512], F32, tag="et", bufs=et_bufs, name="et"
                                )
                                nc.scalar.activation(et[:, rs:], pss[:, rs:], EXP)
                                nc.vector.tensor_mul(
                                    pT[:, rs:],
                                    et[:, rs:],
                                    dmask[:, r * 512 + rs : (r + 1) * 512],
                                )
                            else:
                                nc.scalar.activation(pT[:, :], pss[:, :], EXP)
                            pts[kt] = (pT, rs)

                        def pv(kt):
                            pT, rs = pts.pop(kt)
                            nc.tensor.matmul(
                                pso[:, rs:],
                                vtm[b][:, kt, :],
                                pT[:, rs:],
                                start=(kt == 0),
                                stop=(kt == nkt - 1),
                            )
                            nc.tensor.matmul(
                                psl[0:1, rs:],
                                ones_col[:, :],
                                pT[:, rs:],
                                start=(kt == 0),
                                stop=(kt == nkt - 1),
                            )

                        LA = 2  # scores run this many k-tiles ahead of PV
                        for kt in range(nkt):
                            score(kt)
                            if kt >= LA:
                                pv(kt - LA)
                        for kt in range(max(0, nkt - LA), nkt):
                            pv(kt)

                        linv = ap.tile([1, 512], F32, tag="li", bufs=2, name="linv")
                        nc.vector.reciprocal(linv[:, :], psl[0:1, :])
                        lbc = ap.tile([128, 512], F32, tag="lbc", bufs=2, name="lbc")
                        nc.gpsimd.partition_broadcast(lbc[:, :], linv[0:1, :])
                        aout = ap.tile([128, 512], BF16, tag="ao", bufs=2, name="aout")
                        nc.vector.tensor_mul(aout[:, :], pso[:, :], lbc[:, :])
                        for half in range(2):
                            j = 2 * qb + half
                            nc.sync.dma_start(
                                out=send[b][h][j, :, :],
                                in_=aout[:, half * 256 : (half + 1) * 256],
                            )
                        if fi < len(fill) and it_idx >= fill_from:
                            fill[fi]()
                            fi += 1
                    nc.gpsimd.collective_compute(
                        "AllToAll",
                        mybir.AluOpType.bypass,
                        replica_groups=[list(range(CORES))],
                        ins=[send[b][h].opt()],
                        outs=[recv[b][h].opt()],
                    )
                for i in range(fi, len(fill)):
                    fill[i]()

            # ---- attention, batch 0 then batch 1 ----
            with tc.tile_pool(name="wop", bufs=1) as wop:
                wopre_holder["t"] = wop.tile([128, 2, 16, 512], BF16, name="wopre")

                def emit_wo_prefetch():
                    for nb in range(2):
                        nc.sync.dma_start(
                            out=wopre_holder["t"][:, nb, :, :],
                            in_=wo_e[:, nb * 512 : (nb + 1) * 512].rearrange(
                                "(k p) c -> p k c", p=128
                            ),
                        )

                with (
                    tc.tile_pool(name="ap0", bufs=2, space="PSUM") as apsum0,
                    tc.tile_pool(name="ab0", bufs=1) as ab0,
                ):
                    emit_attention(0, ab0, apsum0)
                emit_wo_prefetch()

                with (
                    tc.tile_pool(name="ap1", bufs=2, space="PSUM") as apsum1,
                    tc.tile_pool(name="ab1", bufs=1) as ab1,
                    tc.tile_pool(name="op0", bufs=1) as op0,
                    tc.tile_pool(name="ops0", bufs=1, space="PSUM") as ops0,
                ):
                    ems = emit_oproj(0, op0, ops0, psum_bufs=1)
                    emit_attention(1, ab1, apsum1, fillers=ems, fill_from=3)

                # ---- o-projection for batch 1 ----
                with (
                    tc.tile_pool(name="op1", bufs=1) as op1,
                    tc.tile_pool(name="ops1p", bufs=1, space="PSUM") as ops1,
                ):
                    for em in emit_oproj(1, op1, ops1, use_prefetch=True):
                        em()

            qkv.__exit__(None, None, None)

    nc.compile()
    return nc


def _host_tables():
    inv_freq = 1.0 / (ROPE_THETA ** (np.arange(0, HD, 2, dtype=np.float64) / HD))
    pos = np.arange(T, dtype=np.float64)
    freqs = pos[:, None] * inv_freq[None, :]  # [T, 64]
    emb = np.concatenate([freqs, freqs], axis=-1)  # [T, 128]
    cosT = np.cos(emb).T.astype(np.float32)  # [128, T]
    sinT = np.sin(emb).T.astype(np.float32)
    sinT[:64, :] *= -1.0  # sign of the rotate-half fold
    # diagonal-band causal masks: dmask[r][k', q'] = 1 if q' >= 128 r + k'
    q = np.arange(512)[None, :]
    kk = np.arange(128)[:, None]
    dm = np.concatenate(
        [(q >= 128 * r + kk).astype(np.float32) for r in range(4)], axis=1
    )  # [128, 2048]
    ident = np.eye(128, dtype=np.float32)
    return cosT, sinT, dm, ident


def _pre_round(a):
    b = np.ascontiguousarray(a, dtype=np.float32).view(np.uint32)
    return ((b + np.uint32(0x800)) & np.uint32(0xFFFFF000)).view(np.float32)


def _run(inputs, trace=False):
    if "nc" not in _CACHE:
        _CACHE["nc"] = _build_nc()
    nc = _CACHE["nc"]

    import ml_dtypes

    x = _pre_round(np.asarray(inputs["x"], dtype=np.float32).reshape(TOK, H))
    wq = _pre_round(np.asarray(inputs["wq"], dtype=np.float32) * np.float32(SCALE))
    wk = _pre_round(np.asarray(inputs["wk"], dtype=np.float32))
    wv = _pre_round(np.asarray(inputs["wv"], dtype=np.float32))
    wo = np.asarray(inputs["wo"], dtype=np.float32).astype(ml_dtypes.bfloat16)
    cosT, sinT, dm, ident = _host_tables()

    in_maps = []
    for c in range(CORES):
        kv = c // 2
        in_maps.append(
            {
                "x": x,
                "wq": np.ascontiguousarray(wq[:, c * DPC : (c + 1) * DPC]),
                "wk": np.ascontiguousarray(wk[:, kv * HD : (kv + 1) * HD]),
                "wv": np.ascontiguousarray(wv[:, kv * HD : (kv + 1) * HD]),
                "wo": wo,
                "cosT": cosT,
                "sinT": sinT,
                "dmask": dm,
                "ident": ident,
            }
        )

    res = run_bass_kernel_spmd(nc, in_maps, core_ids=list(range(CORES)), trace=trace)
    out = np.empty((B, T, H), dtype=np.float32)
    for c in range(CORES):
        o = res.results[c]["out"]  # [B, OWN, H]
        for b in range(B):
            out[b, c * OWN : (c + 1) * OWN, :] = o[b]
    return out, res


def _run_subprocess(inputs):
    """Fresh-process fallback for transient device faults."""
    import os
    import subprocess
    import tempfile

    d = tempfile.mkdtemp()
    inp = os.path.join(d, "in.npz")
    outp = os.path.join(d, "out.npy")
    np.savez(inp, **{k: np.asarray(v) for k, v in inputs.items()})
    code = (
        "import sys, numpy as np; "
        f"sys.path.insert(0, {os.path.dirname(os.path.abspath(__file__))!r}); "
        "import kernel as K; "
        f"d = np.load({inp!r}); "
        "out = K.kernel(**{k: d[k] for k in d.files}); "
        f"np.save({outp!r}, out)"
    )
    subprocess.run([sys.executable, "-c", code], check=True, timeout=900)
    return np.load(outp)


def kernel(**inputs) -> np.ndarray:
    try:
        out, _ = _run(inputs, trace=False)
        return out
    except Exception:
        pass
    # transient accelerator fault: retry in-process once, then isolate
    try:
        out, _ = _run(inputs, trace=False)
        return out
    except Exception:
        pass
    for attempt in range(2):
        try:
            return _run_subprocess(inputs)
        except Exception:
            if attempt == 1:
                raise
    raise RuntimeError("unreachable")
